# revision 6
# baseline (speedup 1.0000x reference)
"""AttentionPooling (segment softmax + weighted segment-sum) on 8 TRN2 cores.

Math per graph g:  out[g,:] = sum_{n in g} softmax_g(x@q)[n] * x[n,:]

Device algorithm (per core, SPMD over an exact 8-way node split):
  nodes are processed in 128-node chunks; blocks of 4096 nodes accumulate
  into a PSUM window of WMAX graph columns (the batch ids are sorted, so a
  4096-node block spans only ~33 graphs).  Per chunk:
    scores  s[n]   = sum_c X[n,c]*q[c]        (DVE tensor_tensor_reduce)
    ex[n]          = exp(s[n])                (ACT; softmax is shift-invariant
                                               and |s| < ~2, so no max pass)
    W[n,j]         = (iota[j]==bl[n]) * ex[n] (GpSimd tensor_scalar dual-op)
    pool[c,j]     += X^T @ W                  (PE matmul, PSUM accumulate)
    ssum[j]       += ones^T @ W               (PE matmul)
  bl[n] = batch[n] - batch[block_start] is precomputed on host (O(N)).

Host combines the per-block partial windows (graphs straddling block/core
boundaries simply get their partials summed) and normalizes: out = pool/ssum.
"""

import os
import sys
from contextlib import ExitStack

import numpy as np

N = 1048576
C = 128
B = 8192
N_CORES = 8
P = 128  # SBUF partitions == nodes per chunk

# (block_nodes, wmax): psum window width must cover the max graph span of any
# block; chosen adaptively at run time from this list.
_CONFIGS = [(4096, 40), (2048, 24), (1024, 16)]
_SUP = 16  # chunks per DMA supertile (16*128 nodes * 512B = 1 MiB per DMA)

_prog_cache: dict = {}
LAST_RUN = None  # BassKernelResults of the most recent device run (for test.py)


def _build_program(n_local: int, block_nodes: int, wmax: int, sup: int):
    import concourse.bass as bass
    import concourse.mybir as mybir
    import concourse.tile as tile
    from concourse import bacc

    f32 = mybir.dt.float32
    n_chunks = n_local // P
    cpb = block_nodes // P  # chunks per block
    n_blocks = n_chunks // cpb
    assert n_local % P == 0 and n_chunks % cpb == 0
    assert cpb % sup == 0 or sup == cpb
    sup_per_block = cpb // sup

    nc = bacc.Bacc("TRN2", target_bir_lowering=False, debug=False)
    x_h = nc.dram_tensor("x", [n_local, C], f32, kind="ExternalInput")
    bl_h = nc.dram_tensor("bl", [P, n_chunks], f32, kind="ExternalInput")
    q_h = nc.dram_tensor("q", [1, C], f32, kind="ExternalInput")
    pool_h = nc.dram_tensor("pool", [P, n_blocks * wmax], f32, kind="ExternalOutput")
    ssum_h = nc.dram_tensor("ssum", [1, n_blocks * wmax], f32, kind="ExternalOutput")

    # x[t*128+p, c] viewed as [p, t, c]: chunk t lands on partitions with one
    # contiguous 512B row per partition.
    x_ap = x_h.ap().rearrange("(t p) c -> p t c", p=P)

    mult = mybir.AluOpType.mult
    add = mybir.AluOpType.add
    is_equal = mybir.AluOpType.is_equal

    with tile.TileContext(nc) as tc, ExitStack() as ctx:
        const = ctx.enter_context(tc.tile_pool(name="const", bufs=1))
        xpool = ctx.enter_context(tc.tile_pool(name="xt", bufs=3))
        spool = ctx.enter_context(tc.tile_pool(name="scr", bufs=4))
        wpool = ctx.enter_context(tc.tile_pool(name="w", bufs=4))
        ppool = ctx.enter_context(tc.tile_pool(name="pp", bufs=2, space="PSUM"))
        sspool = ctx.enter_context(tc.tile_pool(name="ssp", bufs=2, space="PSUM"))

        # --- constants ---
        qrow = const.tile([1, C], f32)
        nc.sync.dma_start(qrow[:], q_h.ap())
        qb = const.tile([P, C], f32)
        nc.gpsimd.partition_broadcast(qb[:], qrow[:])
        iota_i = const.tile([P, wmax], mybir.dt.int32)
        nc.gpsimd.iota(iota_i[:], pattern=[[1, wmax]], base=0, channel_multiplier=0)
        iota_f = const.tile([P, wmax], f32)
        nc.vector.tensor_copy(iota_f[:], iota_i[:])
        ones_t = const.tile([P, 1], f32)
        nc.vector.memset(ones_t[:], 1.0)
        bl_sb = const.tile([P, n_chunks], f32)
        nc.sync.dma_start(bl_sb[:], bl_h.ap())

        s_sb = const.tile([P, n_chunks], f32)
        ex_sb = const.tile([P, n_chunks], f32)
        pstage = const.tile([P, n_blocks * wmax], f32)
        sstage = const.tile([1, n_blocks * wmax], f32)

        for blk in range(n_blocks):
            pp = ppool.tile([P, wmax], f32)
            ssp = sspool.tile([1, wmax], f32)
            for st in range(sup_per_block):
                st_glob = blk * sup_per_block + st
                c0 = st_glob * sup
                xt = xpool.tile([P, sup * C], f32)
                nc.sync.dma_start(
                    xt[:].rearrange("p (t c) -> p t c", c=C),
                    x_ap[:, c0 : c0 + sup, :],
                )
                for i in range(sup):
                    c = c0 + i
                    scr = spool.tile([P, C], f32)
                    # out = (X + 0) * qb ; accum_out = per-node dot product
                    nc.vector.scalar_tensor_tensor(
                        out=scr[:],
                        in0=xt[:, i * C : (i + 1) * C],
                        scalar=0.0,
                        in1=qb[:],
                        op0=add,
                        op1=mult,
                        accum_out=s_sb[:, c : c + 1],
                    )
                nc.scalar.activation(
                    ex_sb[:, c0 : c0 + sup],
                    s_sb[:, c0 : c0 + sup],
                    mybir.ActivationFunctionType.Exp,
                )
                for i in range(sup):
                    c = c0 + i
                    w = wpool.tile([P, wmax], f32)
                    nc.gpsimd.tensor_scalar(
                        w[:],
                        iota_f[:],
                        bl_sb[:, c : c + 1],
                        ex_sb[:, c : c + 1],
                        op0=is_equal,
                        op1=mult,
                    )
                    first = c % cpb == 0
                    last = c % cpb == cpb - 1
                    nc.tensor.matmul(
                        pp[:],
                        lhsT=xt[:, i * C : (i + 1) * C],
                        rhs=w[:],
                        start=first,
                        stop=last,
                    )
                    nc.tensor.matmul(
                        ssp[:], lhsT=ones_t[:], rhs=w[:], start=first, stop=last
                    )
            nc.scalar.copy(pstage[:, blk * wmax : (blk + 1) * wmax], pp[:])
            nc.scalar.copy(sstage[:, blk * wmax : (blk + 1) * wmax], ssp[:])

        nc.sync.dma_start(pool_h.ap(), pstage[:])
        nc.sync.dma_start(ssum_h.ap(), sstage[:])

    nc.compile()
    return nc


def _get_program(n_local: int, block_nodes: int, wmax: int, sup: int):
    key = (n_local, block_nodes, wmax, sup)
    if key not in _prog_cache:
        _prog_cache[key] = _build_program(n_local, block_nodes, wmax, sup)
    return _prog_cache[key]


def _host_prep(batch: np.ndarray, block_nodes: int):
    """Per-node block-local graph ids + per-block base graph ids."""
    n_blocks_g = batch.shape[0] // block_nodes
    bases = batch[:: block_nodes].copy()  # [n_blocks_g]
    spans = batch[block_nodes - 1 :: block_nodes] - bases + 1
    bl = (batch - np.repeat(bases, block_nodes)).astype(np.float32)
    return bases, int(spans.max()), bl


def kernel(x, query, batch, num_graphs):
    x = np.ascontiguousarray(np.asarray(x, dtype=np.float32))
    query = np.asarray(query, dtype=np.float32).reshape(-1)
    batch = np.asarray(batch).astype(np.int64)
    b_total = int(num_graphs)
    n, c = x.shape
    assert n == N and c == C and b_total == B and batch.shape[0] == N

    # pick the largest block size whose max graph span fits the psum window
    for block_nodes, wmax in _CONFIGS:
        bases, max_span, bl = _host_prep(batch, block_nodes)
        if max_span <= wmax:
            break
    else:
        # pathological batch distribution: dense numpy fallback
        return _numpy_reference(x, query, batch, b_total)

    n_local = N // N_CORES
    n_chunks = n_local // P
    nc = _get_program(n_local, block_nodes, wmax, _SUP)

    qrow = np.ascontiguousarray(query.reshape(1, C))
    in_maps = []
    for k in range(N_CORES):
        sl = slice(k * n_local, (k + 1) * n_local)
        bl_k = np.ascontiguousarray(bl[sl].reshape(n_chunks, P).T)
        in_maps.append({"x": x[sl], "bl": bl_k, "q": qrow})

    from concourse.bass_utils import run_bass_kernel_spmd

    kres = run_bass_kernel_spmd(nc, in_maps, core_ids=list(range(N_CORES)))
    global LAST_RUN
    LAST_RUN = kres
    results = kres.results

    # --- host combine: scatter-add block windows, then normalize ---
    n_blocks = n_chunks // (block_nodes // P)
    pool_t = np.zeros((C, b_total), dtype=np.float32)
    ssum = np.zeros(b_total, dtype=np.float32)
    for k in range(N_CORES):
        pool_k = results[k]["pool"].reshape(C, n_blocks, wmax)
        ssum_k = results[k]["ssum"].reshape(n_blocks, wmax)
        for j in range(n_blocks):
            g0 = int(bases[k * n_blocks + j])
            w = min(wmax, b_total - g0)
            pool_t[:, g0 : g0 + w] += pool_k[:, j, :w]
            ssum[g0 : g0 + w] += ssum_k[j, :w]
    out = (pool_t / ssum[None, :]).T
    return np.ascontiguousarray(out.astype(np.float32))


def _numpy_reference(x, query, batch, num_graphs):
    scores = x @ query
    m = np.full(num_graphs, -np.inf, dtype=np.float32)
    np.maximum.at(m, batch, scores)
    ex = np.exp(scores - m[batch])
    s = np.zeros(num_graphs, dtype=np.float32)
    np.add.at(s, batch, ex)
    w = ex / s[batch]
    out = np.zeros((num_graphs, x.shape[1]), dtype=np.float32)
    np.add.at(out, batch, w[:, None] * x)
    return out
